# revision 3
# baseline (speedup 1.0000x reference)
"""MoE cosine-router kernel for 8 Trainium2 NeuronCores.

Reference computation (see problem):
    h      = normalize(x @ W_proj + b_proj, axis=1)          [N, H]
    logits = (h @ normalize(sim_matrix, axis=0)) * exp(min(temperature, log 100))
    expert_mask = one-hot scatter of top-8(logits)           [N, E]
    route_prob  = softmax(logits, -1)                        [N, E]
    importance  = route_prob.sum(0); load = expert_mask.sum(0)

Strategy (data-parallel over N across 8 cores):
  Host:   simn = normalize(sim) ; C = W @ simn (fp32-grade, [D, E]) ;
          bl = b @ simn ; logits_pre = x @ C + bl  (device, fp32 matmuls)
          The row norm ||x@W+b|| only *scales* each logit row by a positive
          scalar, so the top-k mask is unaffected by its precision; softmax
          tolerates ~1e-4.  Hence the big [N,D]@[D,H] matmul runs in bf16
          (full PE rate) purely to obtain row sums of squares, while the
          small [N,D]@[D,E] logits matmul runs in true fp32.
  Device: PE-transpose x tiles (fp32), mm1 in bf16 -> h^T -> Square+bias ->
          sumsq via ones-matmul; mm2 in fp32 -> logits; scale rows by
          exp(temp)/||h||; iterative 8x (reduce_max, is_ge, mask-out) top-k;
          softmax via Exp + reduce + reciprocal.
  Host:   gather shards; importance/load = column sums of the full outputs.
"""

import numpy as np

N_CORES = 8
N, D, H, E = 16384, 4096, 1024, 64
K_TOP = 8
CLAMP_MAX = float(np.log(100.0))
EPS = 1e-12

_CACHE = {}


def _build(R, D_, H_, E_, RC, inv_scale_sq):
    """Build the per-core Tile program. R rows/core, chunked by RC rows."""
    from contextlib import ExitStack

    import concourse.bacc as bacc
    import concourse.mybir as mybir
    import concourse.tile as tile

    f32 = mybir.dt.float32
    bf16 = mybir.dt.bfloat16
    AF = mybir.ActivationFunctionType
    ALU = mybir.AluOpType
    AX = mybir.AxisListType

    KT = D_ // 128          # contraction tiles over D
    MT = H_ // 128          # H tiles
    NRC = R // RC           # row chunks
    RSUB = RC // 128        # 128-row tiles per chunk
    G = R // 128            # total 128-row tiles per core

    nc = bacc.Bacc(None, target_bir_lowering=False, debug=False)
    xs = nc.dram_tensor("xs", [R, D_], f32, kind="ExternalInput")
    wf = nc.dram_tensor("wf", [D_, H_], f32, kind="ExternalInput")
    cc = nc.dram_tensor("cc", [D_, E_], f32, kind="ExternalInput")
    bt_d = nc.dram_tensor("bt", [128, MT], f32, kind="ExternalInput")
    blr_d = nc.dram_tensor("blrep", [128, E_], f32, kind="ExternalInput")
    id_d = nc.dram_tensor("ident", [128, 128], f32, kind="ExternalInput")
    mask_o = nc.dram_tensor("mask_o", [R, E_], f32, kind="ExternalOutput")
    prob_o = nc.dram_tensor("prob_o", [R, E_], f32, kind="ExternalOutput")

    with tile.TileContext(nc) as tc, ExitStack() as ctx:
        const = ctx.enter_context(tc.tile_pool(name="const", bufs=1))
        stage = ctx.enter_context(tc.tile_pool(name="stage", bufs=8))
        wpool = ctx.enter_context(tc.tile_pool(name="wpool", bufs=2))
        xt32p = ctx.enter_context(tc.tile_pool(name="xt32", bufs=1))
        xt16p = ctx.enter_context(tc.tile_pool(name="xt16", bufs=1))
        h2p = ctx.enter_context(tc.tile_pool(name="h2", bufs=2))
        bigp = ctx.enter_context(tc.tile_pool(name="big", bufs=1))
        pT = ctx.enter_context(tc.tile_pool(name="pT", bufs=2, space="PSUM"))
        pH = ctx.enter_context(tc.tile_pool(name="pH", bufs=2, space="PSUM"))
        pL = ctx.enter_context(tc.tile_pool(name="pL", bufs=2, space="PSUM"))
        pN = ctx.enter_context(tc.tile_pool(name="pN", bufs=2, space="PSUM"))

        C32 = const.tile([128, KT, E_], f32)
        nc.sync.dma_start(C32[:], cc.rearrange("(a p) e -> p a e", p=128))
        btile = const.tile([128, MT], f32)
        nc.sync.dma_start(btile[:], bt_d[:])
        blr = const.tile([128, E_], f32)
        nc.sync.dma_start(blr[:], blr_d[:])
        ident = const.tile([128, 128], f32)
        nc.sync.dma_start(ident[:], id_d[:])
        ones16 = const.tile([128, 1], bf16)
        nc.vector.memset(ones16[:], 1.0)

        logit_st = bigp.tile([128, G, E_], f32)
        work = bigp.tile([128, G, E_], f32)
        ge = bigp.tile([128, G, E_], f32)
        ex = bigp.tile([128, G, E_], f32)
        maskt = bigp.tile([128, G, E_], f32)
        probst = bigp.tile([128, G, E_], f32)
        nrm = bigp.tile([128, G], f32)
        srt = bigp.tile([128, G], f32)
        sre = bigp.tile([128, G], f32)
        rmax = bigp.tile([128, G], f32)
        ssum = bigp.tile([128, G], f32)
        rec = bigp.tile([128, G], f32)

        for rc in range(NRC):
            xT32 = xt32p.tile([128, KT, RC], f32, tag="xT32")
            xT16 = xt16p.tile([128, KT, RC], bf16, tag="xT16")
            # ---- T phase: load + PE-transpose x chunk ----
            for k in range(KT):
                st = stage.tile([128, RSUB, 128], f32, tag="st")
                nc.sync.dma_start(
                    st[:],
                    xs[rc * RC:(rc + 1) * RC, k * 128:(k + 1) * 128]
                    .rearrange("(a p) d -> p a d", p=128),
                )
                pt = pT.tile([128, RC], f32, tag="pt")
                for rs in range(RSUB):
                    nc.tensor.transpose(
                        pt[:, rs * 128:(rs + 1) * 128], st[:, rs, :], ident[:]
                    )
                if k % 2 == 0:
                    nc.vector.tensor_copy(xT32[:, k, :], pt[:])
                else:
                    nc.scalar.copy(xT32[:, k, :], pt[:])
                nc.vector.tensor_copy(xT16[:, k, :], xT32[:, k, :])
            # ---- M phase: h^T = W^T x^T in bf16, squared (+bias) to h2 ----
            h2t = h2p.tile([128, MT, RC], bf16, tag="h2t")
            for m in range(MT):
                wt = wpool.tile([128, KT, 128], bf16, tag="wt")
                nc.gpsimd.dma_start(
                    wt[:],
                    wf[:, m * 128:(m + 1) * 128]
                    .rearrange("(a p) c -> p a c", p=128),
                )
                ph = pH.tile([128, RC], f32, tag="ph")
                for k in range(KT):
                    nc.tensor.matmul(
                        ph[:], wt[:, k, :], xT16[:, k, :],
                        start=(k == 0), stop=(k == KT - 1),
                    )
                nc.scalar.activation(
                    h2t[:, m, :], ph[:], AF.Square, bias=btile[:, m:m + 1]
                )
            # ---- P phase: row sumsq, scale, fp32 logits ----
            for rt in range(RSUB):
                g = rc * RSUB + rt
                pn = pN.tile([128, 1], f32, tag="pn")
                for m in range(MT):
                    nc.tensor.matmul(
                        pn[:], h2t[:, m, rt * 128:(rt + 1) * 128], ones16[:],
                        start=(m == 0), stop=(m == MT - 1),
                    )
                nc.vector.tensor_copy(nrm[:, g:g + 1], pn[:])
            sl = slice(rc * RSUB, (rc + 1) * RSUB)
            # srt = sqrt(sumsq) / logit_scale ; sre = logit_scale / sqrt(sumsq)
            nc.scalar.activation(srt[:, sl], nrm[:, sl], AF.Sqrt, scale=inv_scale_sq)
            nc.vector.reciprocal(sre[:, sl], srt[:, sl])
            for rt in range(RSUB):
                g = rc * RSUB + rt
                pl = pL.tile([128, E_], f32, tag="pl")
                for k in range(KT):
                    nc.tensor.matmul(
                        pl[:], xT32[:, k, rt * 128:(rt + 1) * 128], C32[:, k, :],
                        start=(k == 0), stop=(k == KT - 1),
                    )
                nc.vector.tensor_add(pl[:], pl[:], blr[:])
                nc.vector.tensor_scalar_mul(logit_st[:, g, :], pl[:], sre[:, g:g + 1])

        # ---- top-8 mask over E per row ----
        lflat = logit_st[:].rearrange("p g e -> p (g e)")
        wflat = work[:].rearrange("p g e -> p (g e)")
        gflat = ge[:].rearrange("p g e -> p (g e)")
        nc.vector.tensor_copy(wflat, lflat)
        for _ in range(K_TOP):
            nc.vector.tensor_reduce(rmax[:], work[:], axis=AX.X, op=ALU.max)
            rb = rmax[:].unsqueeze(2).broadcast_to([128, G, E_])
            nc.vector.tensor_tensor(ge[:], work[:], rb, op=ALU.is_ge)
            nc.vector.scalar_tensor_tensor(
                wflat, gflat, -1e30, wflat, op0=ALU.mult, op1=ALU.add
            )
        nc.vector.tensor_scalar(
            maskt[:].rearrange("p g e -> p (g e)"), wflat, -1e20, None,
            op0=ALU.is_lt,
        )
        # ---- softmax over E per row ----
        nc.scalar.activation(
            ex[:].rearrange("p g e -> p (g e)"), lflat, AF.Exp
        )
        nc.vector.tensor_reduce(ssum[:], ex[:], axis=AX.X, op=ALU.add)
        nc.vector.reciprocal(rec[:], ssum[:])
        rcb = rec[:].unsqueeze(2).broadcast_to([128, G, E_])
        nc.vector.tensor_tensor(probst[:], ex[:], rcb, op=ALU.mult)

        nc.sync.dma_start(mask_o.rearrange("(t p) e -> p t e", p=128), maskt[:])
        nc.sync.dma_start(prob_o.rearrange("(t p) e -> p t e", p=128), probst[:])

    nc.compile()
    return nc


def _host_prep(x, W_proj, b_proj, sim_matrix, temperature):
    x = np.ascontiguousarray(np.asarray(x, dtype=np.float32))
    W = np.ascontiguousarray(np.asarray(W_proj, dtype=np.float32))
    b = np.asarray(b_proj, dtype=np.float32)
    sim = np.asarray(sim_matrix, dtype=np.float32)
    temp = np.asarray(temperature, dtype=np.float32)

    # normalize(sim, axis=0) in fp32, mimicking the reference
    nrm = np.sqrt(np.sum(sim.astype(np.float64) ** 2, axis=0))
    nrm = np.maximum(nrm, EPS).astype(np.float32)
    simn = (sim / nrm[None, :]).astype(np.float32)

    logit_scale = np.exp(np.minimum(temp.astype(np.float64), CLAMP_MAX))
    logit_scale = float(np.float32(logit_scale.reshape(())))

    C = (W.astype(np.float64) @ simn.astype(np.float64)).astype(np.float32)
    bl = (b.astype(np.float64) @ simn.astype(np.float64)).astype(np.float32)

    MT = H // 128
    bt = np.ascontiguousarray(b.reshape(MT, 128).T)           # [128, MT]
    blrep = np.ascontiguousarray(np.broadcast_to(bl, (128, E)))  # [128, E]
    ident = np.eye(128, dtype=np.float32)
    return x, W, C, bt, blrep, ident, logit_scale


def kernel(x, W_proj, b_proj, sim_matrix, temperature):
    from concourse.bass_utils import run_bass_kernel_spmd

    x, W, C, bt, blrep, ident, logit_scale = _host_prep(
        x, W_proj, b_proj, sim_matrix, temperature
    )
    R = N // N_CORES

    key = (R, D, H, E, logit_scale)
    if key not in _CACHE:
        _CACHE[key] = _build(R, D, H, E, RC=512,
                             inv_scale_sq=float(1.0 / (logit_scale * logit_scale)))
    nc = _CACHE[key]

    in_maps = []
    for c in range(N_CORES):
        in_maps.append({
            "xs": np.ascontiguousarray(x[c * R:(c + 1) * R]),
            "wf": W,
            "cc": C,
            "bt": bt,
            "blrep": blrep,
            "ident": ident,
        })
    res = run_bass_kernel_spmd(nc, in_maps, core_ids=list(range(N_CORES)))

    mask = np.concatenate([r["mask_o"] for r in res.results], axis=0)
    probs = np.concatenate([r["prob_o"] for r in res.results], axis=0)
    importance = probs.astype(np.float64).sum(axis=0).astype(np.float32)
    load = mask.astype(np.float64).sum(axis=0).astype(np.float32)
    return mask, probs, importance, load
